# revision 38
# baseline (speedup 1.0000x reference)
"""AttentionPool kernel for Trainium2 (8 NeuronCores, Bass/Tile).

Reference computation:
    wf = feats @ W.T
    scores = leaky_relu((att_w * wf).sum(-1))
    weights = segment_softmax(scores)
    mol_fp = segment_sum(weights[:, None] * wf)
    out = MLP(mol_fp)   (Dense 512->170 ssp, 170->56 ssp, 56->1)

Algebraic restructuring (exact in real arithmetic):
    scores = leaky_relu(feats @ v)        with v = W.T @ att_w
    S      = segment_sum(weights * feats)             [n_mol, feat]
    mol_fp = S @ W.T                                  (linearity of W)
This removes the [131072 x 512 x 512] matmul entirely (32x fewer FLOPs).

Softmax is computed without max-subtraction (scores are O(+-6) so exp is
safe) and without per-atom normalization: the one-hot scatter matrix
carries exp(score); S rows are divided by the denominator after segment
summation.

Per-core segment sums use the PE one-hot trick: for a 128-atom chunk,
lhsT[a, m] = exp(score_a) * (seg_a == m), rhs = the feats chunk,
accumulated in PSUM. Each chunk row is laid out on the host as
[F'(0:256) | 1 | F'(256:512) | 1 | pad] (516 wide) and the segment
matmul runs as two N=257 halves, so column 256 of each PSUM half
accumulates the softmax denominator for free.

The attention vector v is folded into the features on the host
(F' = feats * v, fp16), which turns the per-atom score into a plain row
sum (split between DVE tensor_reduce and ScalarE activation-accumulate);
v is divided back out of the mol_fp weights (W.T / v, with |v| clamped
at 1e-4 so the fp16 weight magnitudes stay bounded; the clamp cancels
exactly in mol_fp and perturbs scores by <= ~1e-4).

Precision: feats / one-hot / segment-matmul / mol_fp-matmul run in fp16
(single-pass PE matmuls -- fp32 is double-pumped LOW_HIGH = 2x cost;
fp16 keeps 11 mantissa bits: measured end-to-end absmax/scale ~4e-4).
All accumulation (PSUM, score reduction) is fp32, and the MLP runs fp32
(the final output is a near-cancellation of O(1) hidden values, so fp16
noise there would dominate).

Sharding: 512 consecutive molecules per core (segment_ids are sorted),
4 groups of 128 molecules per core. The host pads each group's atom
list to a common capacity C (multiple of 128) with zero feats and
seg = -1 (one-hot never matches => zero contribution).

The shifted-softplus "- log(2)" terms are folded into the downstream
biases on the host: b2' = b2 - log2 * W2.sum(1), b3' = b3 - log2 * W3.sum(1).
Device softplus(x) is computed as ln(exp(x) + 1) (the ACT table set
covering exp has no softplus entry; exp/ln/lrelu/copy share one set).
"""

import os

import numpy as np

N_ATOMS = 131072
N_MOL = 4096
FEAT = 512
N_CORES = 8
MOLS_PER_CORE = N_MOL // N_CORES  # 512
GROUPS_PER_CORE = 4
GROUP_MOLS = 128  # one-hot width / PSUM partition dim
LOG2 = float(np.log(2.0))

# Set by kernel() when tracing is enabled via KERNEL_TRACE=1
LAST_EXEC_NS = None
LAST_RESULTS = None

_PROGRAM_CACHE = {}


def _batches(nchunk, first_group=False):
    """Split nchunk 128-atom chunks into sub-batches of <=9 chunks.
    Each sub-batch is one DMA + one reduce. The first group ramps up with
    small batches so the pipeline fills quickly after launch."""
    if first_group:
        sizes = []
        n = nchunk
        for s in (1, 2, 2, 4):
            if n <= 0:
                break
            s = min(s, n)
            sizes.append(s)
            n -= s
        while n > 0:
            s = min(8, n)
            # avoid a tiny trailing batch
            if 0 < n - s < 3 and s == 8:
                s = n - 2
            sizes.append(s)
            n -= s
        return sizes
    nb = -(-nchunk // 9)
    base = nchunk // nb
    rem = nchunk - base * nb
    return [base + 1] * rem + [base] * (nb - rem)


def _build_program(nchunk):
    import concourse.bacc as bacc
    import concourse.mybir as mybir
    import concourse.tile as tile

    f32 = mybir.dt.float32
    f16 = mybir.dt.float16
    Alu = mybir.AluOpType
    Act = mybir.ActivationFunctionType
    Ax = mybir.AxisListType

    C = nchunk * 128  # atoms per group (padded)
    ncol = GROUPS_PER_CORE * nchunk  # total 128-atom chunks per core
    bsizes = _batches(nchunk)
    bsizes0 = _batches(nchunk, first_group=True)
    BMAX = max(max(bsizes), max(bsizes0))

    nc = bacc.Bacc("TRN2")

    feats_h = nc.dram_tensor("feats", [128, GROUPS_PER_CORE * nchunk, 516], f16, kind="ExternalInput")
    segl_h = nc.dram_tensor("segl", [128, GROUPS_PER_CORE * nchunk], f32, kind="ExternalInput")
    wtp_h = nc.dram_tensor("wtp", [FEAT, FEAT], f16, kind="ExternalInput")
    u1t_h = nc.dram_tensor("u1t", [FEAT, 170], f16, kind="ExternalInput")
    w2t_h = nc.dram_tensor("w2t", [170, 56], f32, kind="ExternalInput")
    w3t_h = nc.dram_tensor("w3t", [56, 1], f32, kind="ExternalInput")
    b1_h = nc.dram_tensor("b1c", [170, 1], f32, kind="ExternalInput")
    b2p_h = nc.dram_tensor("b2p", [56, 1], f32, kind="ExternalInput")
    b3p_h = nc.dram_tensor("b3p", [1, 1], f32, kind="ExternalInput")
    iota_h = nc.dram_tensor("iotaf", [128, 128], f16, kind="ExternalInput")
    id32_h = nc.dram_tensor("ident", [128, 128], f32, kind="ExternalInput")

    molfp_h = nc.dram_tensor("mol_fp", [MOLS_PER_CORE, FEAT], f32, kind="ExternalOutput")
    out_h = nc.dram_tensor("out", [1, MOLS_PER_CORE], f32, kind="ExternalOutput")

    with tile.TileContext(nc) as tc:
        with (
            tc.tile_pool(name="singles", bufs=1) as singles,
            tc.tile_pool(name="fpool", bufs=9) as fpool,
            tc.tile_pool(name="scratch", bufs=2) as scratch,
            tc.tile_pool(name="ohpool", bufs=8) as ohpool,
            tc.tile_pool(name="misc", bufs=3) as misc,
            tc.tile_pool(name="stpool", bufs=5) as stpool,
            tc.tile_pool(name="ps_sa", bufs=2, space="PSUM") as ps_sa,
            tc.tile_pool(name="ps_sb", bufs=2, space="PSUM") as ps_sb,
            tc.tile_pool(name="ps_t", bufs=1, space="PSUM") as ps_t,
            tc.tile_pool(name="ps_m", bufs=1, space="PSUM") as ps_m,
            tc.tile_pool(name="ps_h", bufs=1, space="PSUM") as ps_h,
        ):
            # ---- constants / weights (loaded once) ----
            # tiles for weights/constants; the heavier weight DMAs are
            # emitted after the main loop so the startup Sync queue serves
            # the first feats batches first (deps still pull them in time)
            wtp_sb = singles.tile([128, 4, FEAT], f16)
            u1t_sb = singles.tile([128, 4, 170], f16)
            w2ta = singles.tile([128, 56], f32)
            w2tb = singles.tile([42, 56], f32)
            w3t_sb = singles.tile([56, 1], f32)
            b1a = singles.tile([128, 1], f32)
            b1b = singles.tile([42, 1], f32)
            b2p = singles.tile([56, 1], f32)
            b3p = singles.tile([1, 1], f32)
            id32_sb = singles.tile([128, 128], f32)
            iota_sb = singles.tile([128, 128], f16)
            nc.sync.dma_start(out=iota_sb, in_=iota_h[:, :])
            seg_all = singles.tile([128, ncol], f32)
            nc.sync.dma_start(out=seg_all, in_=segl_h[:, :])

            def load_weights_late():
                nc.sync.dma_start(out=wtp_sb, in_=wtp_h[:, :].rearrange("(c p) j -> p c j", p=128))
                nc.sync.dma_start(out=u1t_sb, in_=u1t_h[:, :].rearrange("(c p) k -> p c k", p=128))
                nc.sync.dma_start(out=w2ta, in_=w2t_h[0:128, :])
                nc.sync.dma_start(out=w2tb, in_=w2t_h[128:170, :])
                nc.sync.dma_start(out=w3t_sb, in_=w3t_h[:, :])
                nc.sync.dma_start(out=b1a, in_=b1_h[0:128, :])
                nc.sync.dma_start(out=b1b, in_=b1_h[128:170, :])
                nc.sync.dma_start(out=b2p, in_=b2p_h[:, :])
                nc.sync.dma_start(out=b3p, in_=b3p_h[:, :])
                nc.sync.dma_start(out=id32_sb, in_=id32_h[:, :])

            scores_all = singles.tile([128, ncol], f32)
            lr_all = singles.tile([128, ncol], f32)
            ex_all = singles.tile([128, ncol], f32)
            # first-MLP-layer accumulators, filled per group directly from
            # the transposed-S tiles via U1 = (W.T/v) @ W1.T folded on host
            H1a = ps_h.tile([128, MOLS_PER_CORE], f32, tag="h1a")
            H1b = ps_h.tile([42, MOLS_PER_CORE], f32, tag="h1b")

            F_slices = {}  # col -> (tile, idx within batch)
            gbatch = 0
            weights_loaded = [False]
            for g in range(GROUPS_PER_CORE):
                # ---- phase 1: stream feats sub-batches; scores = rowsum(F')
                # (features are pre-scaled by v on the host). Reduce work is
                # split between DVE (tensor_reduce, batched) and the Scalar
                # engine (activation Copy with accumulate, per chunk).
                b0 = 0
                for bsz in (bsizes0 if g == 0 else bsizes):
                    col0 = g * nchunk + b0
                    # 516-wide chunk rows: [F(0:256) | 1 | F(256:512) | 1 | pad]
                    # so each half of the segment matmul (N=257) carries the
                    # softmax denominator in its ones column. DRAM layout is
                    # partition-major: one contiguous run per partition per DMA.
                    F = fpool.tile([128, bsz, 516], f16, tag="F")
                    nc.sync.dma_start(out=F, in_=feats_h[:, col0 : col0 + bsz, :])
                    fdat = F[:, :, 0:514].rearrange("p c (h e) -> p c h e", h=2)[
                        :, :, :, 0:256
                    ]
                    if gbatch % 2 == 0 and gbatch != 8:
                        # reduce halves separately (skips the ones columns),
                        # then add the two partial sums
                        part = scratch.tile([128, BMAX, 2], f32, tag="part")
                        nc.vector.tensor_reduce(
                            out=part[:, :bsz, :],
                            in_=fdat,
                            axis=Ax.X,
                            op=Alu.add,
                        )
                        nc.vector.tensor_tensor(
                            out=scores_all[:, col0 : col0 + bsz],
                            in0=part[:, :bsz, 0],
                            in1=part[:, :bsz, 1],
                            op=Alu.add,
                        )
                    else:
                        for j in range(bsz):
                            scr = scratch.tile([128, FEAT], f16, tag="scr")
                            nc.scalar.activation(
                                out=scr,
                                in_=fdat[:, j, :, :],
                                func=Act.Copy,
                                bias=0.0,
                                scale=1.0,
                                accum_out=scores_all[:, col0 + j : col0 + j + 1],
                            )
                    # leaky_relu (DVE, avoids an ACT table swap) + exp per
                    # batch so phase 2 can start before the group completes
                    bb = slice(col0, col0 + bsz)
                    # leaky_relu fused: max(0.01*s, s) in one DVE op
                    nc.vector.scalar_tensor_tensor(
                        out=lr_all[:, bb],
                        in0=scores_all[:, bb],
                        scalar=0.01,
                        in1=scores_all[:, bb],
                        op0=Alu.mult,
                        op1=Alu.max,
                    )
                    nc.scalar.activation(out=ex_all[:, bb], in_=lr_all[:, bb], func=Act.Exp)
                    for j in range(bsz):
                        F_slices[col0 + j] = (F, j)
                    b0 += bsz
                    gbatch += 1
                    if not weights_loaded[0]:
                        weights_loaded[0] = True
                        load_weights_late()

                # ---- phase 2: one-hot segment-sum matmuls (two N=257
                # halves; col 256 of each accumulates the denominator) ----
                S_a = ps_sa.tile([128, 257], f32, tag="sa")
                S_b = ps_sb.tile([128, 257], f32, tag="sb")
                for c in range(nchunk):
                    col = g * nchunk + c
                    oh = ohpool.tile([128, 128], f16, tag="oh")
                    # oh[a, m] = (iota[a, m] == seg[a]) * ex[a]
                    nc.vector.tensor_scalar(
                        out=oh,
                        in0=iota_sb,
                        scalar1=seg_all[:, col : col + 1],
                        scalar2=ex_all[:, col : col + 1],
                        op0=Alu.is_equal,
                        op1=Alu.mult,
                    )
                    Ft, j = F_slices.pop(col)
                    nc.tensor.matmul(
                        S_a, oh, Ft[:, j, 0:257], start=(c == 0), stop=(c == nchunk - 1)
                    )
                    nc.tensor.matmul(
                        S_b, oh, Ft[:, j, 257:514], start=(c == 0), stop=(c == nchunk - 1)
                    )

                # ---- phase 3: normalize, mol_fp = S @ W.T ----
                denc = misc.tile([128, 1], f32, tag="denc")
                nc.vector.tensor_scalar_max(denc, S_a[:, 256:257], 1e-30)
                recip = misc.tile([128, 1], f32, tag="recip")
                nc.vector.reciprocal(recip, denc)
                S_sb = misc.tile([128, FEAT], f32, tag="ssb")
                nc.vector.tensor_scalar_mul(S_sb[:, 0:256], S_a[:, 0:256], recip)
                nc.vector.tensor_scalar_mul(S_sb[:, 256:512], S_b[:, 0:256], recip)

                MF = ps_m.tile([128, FEAT], f32, tag="mf")
                STs = []
                for ic in range(4):
                    STp = ps_t.tile([128, 128], f32, tag="tr")
                    nc.tensor.transpose(STp, S_sb[:, ic * 128 : (ic + 1) * 128], id32_sb)
                    ST = stpool.tile([128, 128], f16, tag="st")
                    nc.scalar.copy(out=ST, in_=STp)
                    STs.append(ST)
                    nc.tensor.matmul(
                        MF, ST, wtp_sb[:, ic, :], start=(ic == 0), stop=(ic == 3)
                    )
                mf_sb = misc.tile([128, FEAT], f32, tag="mfsb")
                nc.vector.tensor_copy(mf_sb, MF)
                nc.sync.dma_start(out=molfp_h[g * 128 : (g + 1) * 128, :], in_=mf_sb)

                # first MLP layer accumulates per group from the ST tiles:
                # H1[:, g-slice] = sum_ic u1t[ic].T @ ST[ic]
                ms = slice(g * 128, (g + 1) * 128)
                for ic in range(4):
                    nc.tensor.matmul(
                        H1a[:, ms], u1t_sb[:, ic, 0:128], STs[ic],
                        start=(ic == 0), stop=(ic == 3),
                    )
                for ic in range(4):
                    nc.tensor.matmul(
                        H1b[:, ms], u1t_sb[:, ic, 128:170], STs[ic],
                        start=(ic == 0), stop=(ic == 3),
                    )

            # ---- MLP over all 512 molecules of this core ----
            # softplus(x + b) computed as ln(exp(x + b) + 1); exp's grouped
            # before ln's to minimize ACT function-table reloads
            t1a = singles.tile([128, MOLS_PER_CORE], f32)
            nc.scalar.activation(out=t1a, in_=H1a, func=Act.Exp, bias=b1a, scale=1.0)
            t1b = singles.tile([42, MOLS_PER_CORE], f32)
            nc.scalar.activation(out=t1b, in_=H1b, func=Act.Exp, bias=b1b, scale=1.0)
            h1a = singles.tile([128, MOLS_PER_CORE], f32)
            nc.scalar.activation(out=h1a, in_=t1a, func=Act.Ln, bias=1.0, scale=1.0)
            h1b = singles.tile([42, MOLS_PER_CORE], f32)
            nc.scalar.activation(out=h1b, in_=t1b, func=Act.Ln, bias=1.0, scale=1.0)

            H2 = ps_m.tile([56, MOLS_PER_CORE], f32, tag="mf")
            nc.tensor.matmul(H2, w2ta, h1a, start=True, stop=False)
            nc.tensor.matmul(H2, w2tb, h1b, start=False, stop=True)
            t2 = singles.tile([56, MOLS_PER_CORE], f32)
            nc.scalar.activation(out=t2, in_=H2, func=Act.Exp, bias=b2p, scale=1.0)
            h2 = singles.tile([56, MOLS_PER_CORE], f32)
            nc.scalar.activation(out=h2, in_=t2, func=Act.Ln, bias=1.0, scale=1.0)

            H3 = ps_m.tile([1, MOLS_PER_CORE], f32, tag="mf")
            nc.tensor.matmul(H3, w3t_sb, h2, start=True, stop=True)
            outs = singles.tile([1, MOLS_PER_CORE], f32)
            nc.vector.tensor_scalar_add(outs, H3, b3p[0:1, 0:1])
            nc.sync.dma_start(out=out_h[:, :], in_=outs)

    nc.compile()
    return nc


def kernel(feats, segment_ids, W, att_w, W1, b1, W2, b2, W3, b3):
    global LAST_EXEC_NS, LAST_RESULTS

    feats = np.asarray(feats, dtype=np.float32)
    segment_ids = np.asarray(segment_ids, dtype=np.int32)
    W = np.asarray(W, dtype=np.float32)
    att_w = np.asarray(att_w, dtype=np.float32)
    W1 = np.asarray(W1, dtype=np.float32)
    b1 = np.asarray(b1, dtype=np.float32)
    W2 = np.asarray(W2, dtype=np.float32)
    b2 = np.asarray(b2, dtype=np.float32)
    W3 = np.asarray(W3, dtype=np.float32)
    b3 = np.asarray(b3, dtype=np.float32)

    # ---- host-side shard + pad (segment_ids are sorted) ----
    counts = np.bincount(segment_ids, minlength=N_MOL)
    starts = np.zeros(N_MOL + 1, dtype=np.int64)
    np.cumsum(counts, out=starts[1:])

    n_groups_total = N_CORES * GROUPS_PER_CORE
    gcounts = counts.reshape(n_groups_total, GROUP_MOLS).sum(axis=1)
    C = int(-(-int(gcounts.max()) // 128) * 128)
    C = max(C, 128)
    nchunk = C // 128
    BMAX = max(max(_batches(nchunk)), max(_batches(nchunk, first_group=True)))

    # shared (replicated) host-prepped weights
    v = (att_w @ W).astype(np.float32)  # v = W.T @ att_w
    # the attention vector v is folded into the features on the host
    # (F' = feats * v_safe); it is divided back out of the mol_fp weights.
    # |v| is clamped so |W.T/v| <= ~450, always fp16-representable; the
    # clamp is used consistently on both sides so it cancels exactly in
    # mol_fp and only perturbs attention scores by <= |F|*1e-4.
    v = np.where(np.abs(v) < 1e-4, np.where(v < 0, -1e-4, 1e-4), v).astype(np.float32)
    wtp32 = (W.T / v[:, None]).astype(np.float32)
    wtp = np.ascontiguousarray(wtp32.astype(np.float16))
    # W1 folded through the mol_fp weights: H1 = (S'/den) @ u1t
    u1t = np.ascontiguousarray((wtp32 @ W1.T).astype(np.float16))
    w2t = np.ascontiguousarray(W2.T)
    w3t = np.ascontiguousarray(W3.T)
    b1c = np.ascontiguousarray(b1.reshape(170, 1))
    b2p = np.ascontiguousarray((b2 - LOG2 * W2.sum(axis=1)).reshape(56, 1)).astype(np.float32)
    b3p = np.ascontiguousarray((b3 - LOG2 * W3.sum(axis=1)).reshape(1, 1)).astype(np.float32)
    iotaf = np.ascontiguousarray(
        np.broadcast_to(np.arange(128, dtype=np.float16), (128, 128))
    )
    ident = np.eye(128, dtype=np.float32)

    in_maps = []
    for core in range(N_CORES):
        # row layout: [F'(0:256) | 1 | F'(256:512) | 1 | pad pad] so each
        # N=257 half of the segment matmul carries a denominator column
        fp = np.zeros((GROUPS_PER_CORE * C, 516), dtype=np.float16)
        fp[:, 256] = 1.0
        fp[:, 513] = 1.0
        sl = np.full((GROUPS_PER_CORE * C,), -1.0, dtype=np.float32)
        for g in range(GROUPS_PER_CORE):
            mol0 = core * MOLS_PER_CORE + g * GROUP_MOLS
            a0 = starts[mol0]
            a1 = starts[mol0 + GROUP_MOLS]
            cnt = a1 - a0
            fpr = (feats[a0:a1] * v[None, :]).astype(np.float16)
            fp[g * C : g * C + cnt, 0:256] = fpr[:, 0:256]
            fp[g * C : g * C + cnt, 257:513] = fpr[:, 256:512]
            sl[g * C : g * C + cnt] = (segment_ids[a0:a1] - mol0).astype(np.float32)
        fp = np.ascontiguousarray(
            fp.reshape(GROUPS_PER_CORE * C // 128, 128, 516).transpose(1, 0, 2)
        )
        sl = np.ascontiguousarray(
            sl.reshape(GROUPS_PER_CORE * C // 128, 128).transpose(1, 0)
        )
        in_maps.append(
            dict(
                feats=fp,
                segl=sl,
                wtp=wtp,
                u1t=u1t,
                w2t=w2t,
                w3t=w3t,
                b1c=b1c,
                b2p=b2p,
                b3p=b3p,
                iotaf=iotaf,
                ident=ident,
            )
        )

    if nchunk not in _PROGRAM_CACHE:
        _PROGRAM_CACHE[nchunk] = _build_program(nchunk)
    nc = _PROGRAM_CACHE[nchunk]

    from concourse.bass_utils import run_bass_kernel_spmd

    trace = os.environ.get("KERNEL_TRACE", "0") == "1"
    res = run_bass_kernel_spmd(
        nc, in_maps, core_ids=list(range(N_CORES)), trace=trace
    )
    LAST_EXEC_NS = res.exec_time_ns
    LAST_RESULTS = res

    mol_fp = np.concatenate([r["mol_fp"] for r in res.results], axis=0)
    out = np.concatenate([r["out"].reshape(-1) for r in res.results], axis=0)
    return out, mol_fp


# revision 39
# speedup vs baseline: 1.0149x; 1.0149x over previous
"""AttentionPool kernel for Trainium2 (8 NeuronCores, Bass/Tile).

Reference computation:
    wf = feats @ W.T
    scores = leaky_relu((att_w * wf).sum(-1))
    weights = segment_softmax(scores)
    mol_fp = segment_sum(weights[:, None] * wf)
    out = MLP(mol_fp)   (Dense 512->170 ssp, 170->56 ssp, 56->1)

Algebraic restructuring (exact in real arithmetic):
    scores = leaky_relu(feats @ v)        with v = W.T @ att_w
    S      = segment_sum(weights * feats)             [n_mol, feat]
    mol_fp = S @ W.T                                  (linearity of W)
This removes the [131072 x 512 x 512] matmul entirely (32x fewer FLOPs).

Softmax is computed without max-subtraction (scores are O(+-6) so exp is
safe) and without per-atom normalization: the one-hot scatter matrix
carries exp(score); S rows are divided by the denominator after segment
summation.

Per-core segment sums use the PE one-hot trick: for a 128-atom chunk,
lhsT[a, m] = exp(score_a) * (seg_a == m), rhs = the feats chunk,
accumulated in PSUM. Each chunk row is laid out on the host as
[F'(0:256) | 1 | F'(256:512) | 1 | pad] (516 wide) and the segment
matmul runs as two N=257 halves, so column 256 of each PSUM half
accumulates the softmax denominator for free.

The attention vector v is folded into the features on the host
(F' = feats * v, fp16), which turns the per-atom score into a plain row
sum (split between DVE tensor_reduce and ScalarE activation-accumulate);
v is divided back out of the mol_fp weights (W.T / v, with |v| clamped
at 1e-4 so the fp16 weight magnitudes stay bounded; the clamp cancels
exactly in mol_fp and perturbs scores by <= ~1e-4).

Precision: feats / one-hot / segment-matmul / mol_fp-matmul run in fp16
(single-pass PE matmuls -- fp32 is double-pumped LOW_HIGH = 2x cost;
fp16 keeps 11 mantissa bits: measured end-to-end absmax/scale ~4e-4).
All accumulation (PSUM, score reduction) is fp32, and the MLP runs fp32
(the final output is a near-cancellation of O(1) hidden values, so fp16
noise there would dominate).

Sharding: 512 consecutive molecules per core (segment_ids are sorted),
4 groups of 128 molecules per core. The host pads each group's atom
list to a common capacity C (multiple of 128) with zero feats and
seg = -1 (one-hot never matches => zero contribution).

The shifted-softplus "- log(2)" terms are folded into the downstream
biases on the host: b2' = b2 - log2 * W2.sum(1), b3' = b3 - log2 * W3.sum(1).
Device softplus(x) is computed as ln(exp(x) + 1) (the ACT table set
covering exp has no softplus entry; exp/ln/lrelu/copy share one set).
"""

import os

import numpy as np

N_ATOMS = 131072
N_MOL = 4096
FEAT = 512
N_CORES = 8
MOLS_PER_CORE = N_MOL // N_CORES  # 512
GROUPS_PER_CORE = 4
GROUP_MOLS = 128  # one-hot width / PSUM partition dim
LOG2 = float(np.log(2.0))

# Set by kernel() when tracing is enabled via KERNEL_TRACE=1
LAST_EXEC_NS = None
LAST_RESULTS = None

_PROGRAM_CACHE = {}


def _batches(nchunk, first_group=False):
    """Split nchunk 128-atom chunks into sub-batches of <=9 chunks.
    Each sub-batch is one DMA + one reduce. The first group ramps up with
    small batches so the pipeline fills quickly after launch."""
    if first_group:
        sizes = []
        n = nchunk
        for s in (1, 2, 2, 4):
            if n <= 0:
                break
            s = min(s, n)
            sizes.append(s)
            n -= s
        while n > 0:
            s = min(8, n)
            # avoid a tiny trailing batch
            if 0 < n - s < 3 and s == 8:
                s = n - 2
            sizes.append(s)
            n -= s
        return sizes
    nb = -(-nchunk // 9)
    base = nchunk // nb
    rem = nchunk - base * nb
    return [base + 1] * rem + [base] * (nb - rem)


def _build_program(nchunk):
    import concourse.bacc as bacc
    import concourse.mybir as mybir
    import concourse.tile as tile

    f32 = mybir.dt.float32
    f16 = mybir.dt.float16
    Alu = mybir.AluOpType
    Act = mybir.ActivationFunctionType
    Ax = mybir.AxisListType

    C = nchunk * 128  # atoms per group (padded)
    ncol = GROUPS_PER_CORE * nchunk  # total 128-atom chunks per core
    bsizes = _batches(nchunk)
    bsizes0 = _batches(nchunk, first_group=True)
    BMAX = max(max(bsizes), max(bsizes0))

    nc = bacc.Bacc("TRN2")

    feats_h = nc.dram_tensor("feats", [128, GROUPS_PER_CORE * nchunk, 516], f16, kind="ExternalInput")
    segl_h = nc.dram_tensor("segl", [128, GROUPS_PER_CORE * nchunk], f32, kind="ExternalInput")
    wtp_h = nc.dram_tensor("wtp", [FEAT, FEAT], f16, kind="ExternalInput")
    u1t_h = nc.dram_tensor("u1t", [FEAT, 170], f16, kind="ExternalInput")
    w2t_h = nc.dram_tensor("w2t", [170, 56], f32, kind="ExternalInput")
    w3t_h = nc.dram_tensor("w3t", [56, 1], f32, kind="ExternalInput")
    b1_h = nc.dram_tensor("b1c", [170, 1], f32, kind="ExternalInput")
    b2p_h = nc.dram_tensor("b2p", [56, 1], f32, kind="ExternalInput")
    b3p_h = nc.dram_tensor("b3p", [1, 1], f32, kind="ExternalInput")
    iota_h = nc.dram_tensor("iotaf", [128, 128], f16, kind="ExternalInput")
    id32_h = nc.dram_tensor("ident", [128, 128], f32, kind="ExternalInput")

    molfp_h = nc.dram_tensor("mol_fp", [MOLS_PER_CORE, FEAT], f32, kind="ExternalOutput")
    out_h = nc.dram_tensor("out", [1, MOLS_PER_CORE], f32, kind="ExternalOutput")

    with tile.TileContext(nc) as tc:
        with (
            tc.tile_pool(name="singles", bufs=1) as singles,
            tc.tile_pool(name="fpool", bufs=9) as fpool,
            tc.tile_pool(name="scratch", bufs=2) as scratch,
            tc.tile_pool(name="ohpool", bufs=8) as ohpool,
            tc.tile_pool(name="misc", bufs=3) as misc,
            tc.tile_pool(name="stpool", bufs=5) as stpool,
            tc.tile_pool(name="ps_sa", bufs=2, space="PSUM") as ps_sa,
            tc.tile_pool(name="ps_sb", bufs=2, space="PSUM") as ps_sb,
            tc.tile_pool(name="ps_t", bufs=1, space="PSUM") as ps_t,
            tc.tile_pool(name="ps_m", bufs=1, space="PSUM") as ps_m,
            tc.tile_pool(name="ps_h", bufs=1, space="PSUM") as ps_h,
        ):
            # ---- constants / weights (loaded once) ----
            # tiles for weights/constants; the heavier weight DMAs are
            # emitted after the main loop so the startup Sync queue serves
            # the first feats batches first (deps still pull them in time)
            wtp_sb = singles.tile([128, 4, FEAT], f16)
            u1t_sb = singles.tile([128, 4, 170], f16)
            w2ta = singles.tile([128, 56], f32)
            w2tb = singles.tile([42, 56], f32)
            w3t_sb = singles.tile([56, 1], f32)
            b1a = singles.tile([128, 1], f32)
            b1b = singles.tile([42, 1], f32)
            b2p = singles.tile([56, 1], f32)
            b3p = singles.tile([1, 1], f32)
            id32_sb = singles.tile([128, 128], f32)
            iota_sb = singles.tile([128, 128], f16)
            nc.sync.dma_start(out=iota_sb, in_=iota_h[:, :])
            seg_all = singles.tile([128, ncol], f32)
            nc.sync.dma_start(out=seg_all, in_=segl_h[:, :])

            def load_weights_late():
                nc.sync.dma_start(out=wtp_sb, in_=wtp_h[:, :].rearrange("(c p) j -> p c j", p=128))
                nc.sync.dma_start(out=u1t_sb, in_=u1t_h[:, :].rearrange("(c p) k -> p c k", p=128))
                nc.sync.dma_start(out=w2ta, in_=w2t_h[0:128, :])
                nc.sync.dma_start(out=w2tb, in_=w2t_h[128:170, :])
                nc.sync.dma_start(out=w3t_sb, in_=w3t_h[:, :])
                nc.sync.dma_start(out=b1a, in_=b1_h[0:128, :])
                nc.sync.dma_start(out=b1b, in_=b1_h[128:170, :])
                nc.sync.dma_start(out=b2p, in_=b2p_h[:, :])
                nc.sync.dma_start(out=b3p, in_=b3p_h[:, :])
                nc.sync.dma_start(out=id32_sb, in_=id32_h[:, :])

            scores_all = singles.tile([128, ncol], f32)
            lr_all = singles.tile([128, ncol], f32)
            ex_all = singles.tile([128, ncol], f32)
            # first-MLP-layer accumulators, filled per group directly from
            # the transposed-S tiles via U1 = (W.T/v) @ W1.T folded on host
            H1a = ps_h.tile([128, MOLS_PER_CORE], f32, tag="h1a")
            H1b = ps_h.tile([42, MOLS_PER_CORE], f32, tag="h1b")

            F_slices = {}  # col -> (tile, idx within batch)
            gbatch = 0
            weights_loaded = [False]
            for g in range(GROUPS_PER_CORE):
                # ---- phase 1: stream feats sub-batches; scores = rowsum(F')
                # (features are pre-scaled by v on the host). Reduce work is
                # split between DVE (tensor_reduce, batched) and the Scalar
                # engine (activation Copy with accumulate, per chunk).
                b0 = 0
                for bsz in (bsizes0 if g == 0 else bsizes):
                    col0 = g * nchunk + b0
                    # 516-wide chunk rows: [F(0:256) | 1 | F(256:512) | 1 | pad]
                    # so each half of the segment matmul (N=257) carries the
                    # softmax denominator in its ones column. DRAM layout is
                    # partition-major: one contiguous run per partition per DMA.
                    F = fpool.tile([128, bsz, 516], f16, tag="F")
                    nc.sync.dma_start(out=F, in_=feats_h[:, col0 : col0 + bsz, :])
                    fdat = F[:, :, 0:514].rearrange("p c (h e) -> p c h e", h=2)[
                        :, :, :, 0:256
                    ]
                    # split each batch's reduction DVE/ACT so both engines
                    # work concurrently -- keeps the serial latency before
                    # this batch's exp (and the PE's one-hot supply) short
                    h = (bsz + 1) // 2
                    part = scratch.tile([128, BMAX, 2], f32, tag="part")
                    nc.vector.tensor_reduce(
                        out=part[:, :h, :],
                        in_=fdat[:, :h, :, :],
                        axis=Ax.X,
                        op=Alu.add,
                    )
                    nc.vector.tensor_tensor(
                        out=scores_all[:, col0 : col0 + h],
                        in0=part[:, :h, 0],
                        in1=part[:, :h, 1],
                        op=Alu.add,
                    )
                    for j in range(h, bsz):
                        scr = scratch.tile([128, FEAT], f16, tag="scr")
                        nc.scalar.activation(
                            out=scr,
                            in_=fdat[:, j, :, :],
                            func=Act.Copy,
                            bias=0.0,
                            scale=1.0,
                            accum_out=scores_all[:, col0 + j : col0 + j + 1],
                        )
                    # leaky_relu (DVE, avoids an ACT table swap) + exp per
                    # batch so phase 2 can start before the group completes
                    bb = slice(col0, col0 + bsz)
                    # leaky_relu fused: max(0.01*s, s) in one DVE op
                    nc.vector.scalar_tensor_tensor(
                        out=lr_all[:, bb],
                        in0=scores_all[:, bb],
                        scalar=0.01,
                        in1=scores_all[:, bb],
                        op0=Alu.mult,
                        op1=Alu.max,
                    )
                    nc.scalar.activation(out=ex_all[:, bb], in_=lr_all[:, bb], func=Act.Exp)
                    for j in range(bsz):
                        F_slices[col0 + j] = (F, j)
                    b0 += bsz
                    gbatch += 1
                    if not weights_loaded[0]:
                        weights_loaded[0] = True
                        load_weights_late()

                # ---- phase 2: one-hot segment-sum matmuls (two N=257
                # halves; col 256 of each accumulates the denominator) ----
                S_a = ps_sa.tile([128, 257], f32, tag="sa")
                S_b = ps_sb.tile([128, 257], f32, tag="sb")
                for c in range(nchunk):
                    col = g * nchunk + c
                    oh = ohpool.tile([128, 128], f16, tag="oh")
                    # oh[a, m] = (iota[a, m] == seg[a]) * ex[a]
                    nc.vector.tensor_scalar(
                        out=oh,
                        in0=iota_sb,
                        scalar1=seg_all[:, col : col + 1],
                        scalar2=ex_all[:, col : col + 1],
                        op0=Alu.is_equal,
                        op1=Alu.mult,
                    )
                    Ft, j = F_slices.pop(col)
                    nc.tensor.matmul(
                        S_a, oh, Ft[:, j, 0:257], start=(c == 0), stop=(c == nchunk - 1)
                    )
                    nc.tensor.matmul(
                        S_b, oh, Ft[:, j, 257:514], start=(c == 0), stop=(c == nchunk - 1)
                    )

                # ---- phase 3: normalize, mol_fp = S @ W.T ----
                denc = misc.tile([128, 1], f32, tag="denc")
                nc.vector.tensor_scalar_max(denc, S_a[:, 256:257], 1e-30)
                recip = misc.tile([128, 1], f32, tag="recip")
                nc.vector.reciprocal(recip, denc)
                S_sb = misc.tile([128, FEAT], f32, tag="ssb")
                nc.vector.tensor_scalar_mul(S_sb[:, 0:256], S_a[:, 0:256], recip)
                nc.vector.tensor_scalar_mul(S_sb[:, 256:512], S_b[:, 0:256], recip)

                MF = ps_m.tile([128, FEAT], f32, tag="mf")
                STs = []
                for ic in range(4):
                    STp = ps_t.tile([128, 128], f32, tag="tr")
                    nc.tensor.transpose(STp, S_sb[:, ic * 128 : (ic + 1) * 128], id32_sb)
                    ST = stpool.tile([128, 128], f16, tag="st")
                    nc.scalar.copy(out=ST, in_=STp)
                    STs.append(ST)
                    nc.tensor.matmul(
                        MF, ST, wtp_sb[:, ic, :], start=(ic == 0), stop=(ic == 3)
                    )
                mf_sb = misc.tile([128, FEAT], f32, tag="mfsb")
                nc.vector.tensor_copy(mf_sb, MF)
                nc.sync.dma_start(out=molfp_h[g * 128 : (g + 1) * 128, :], in_=mf_sb)

                # first MLP layer accumulates per group from the ST tiles:
                # H1[:, g-slice] = sum_ic u1t[ic].T @ ST[ic]
                ms = slice(g * 128, (g + 1) * 128)
                for ic in range(4):
                    nc.tensor.matmul(
                        H1a[:, ms], u1t_sb[:, ic, 0:128], STs[ic],
                        start=(ic == 0), stop=(ic == 3),
                    )
                for ic in range(4):
                    nc.tensor.matmul(
                        H1b[:, ms], u1t_sb[:, ic, 128:170], STs[ic],
                        start=(ic == 0), stop=(ic == 3),
                    )

            # ---- MLP over all 512 molecules of this core ----
            # softplus(x + b) computed as ln(exp(x + b) + 1); exp's grouped
            # before ln's to minimize ACT function-table reloads
            t1a = singles.tile([128, MOLS_PER_CORE], f32)
            nc.scalar.activation(out=t1a, in_=H1a, func=Act.Exp, bias=b1a, scale=1.0)
            t1b = singles.tile([42, MOLS_PER_CORE], f32)
            nc.scalar.activation(out=t1b, in_=H1b, func=Act.Exp, bias=b1b, scale=1.0)
            h1a = singles.tile([128, MOLS_PER_CORE], f32)
            nc.scalar.activation(out=h1a, in_=t1a, func=Act.Ln, bias=1.0, scale=1.0)
            h1b = singles.tile([42, MOLS_PER_CORE], f32)
            nc.scalar.activation(out=h1b, in_=t1b, func=Act.Ln, bias=1.0, scale=1.0)

            H2 = ps_m.tile([56, MOLS_PER_CORE], f32, tag="mf")
            nc.tensor.matmul(H2, w2ta, h1a, start=True, stop=False)
            nc.tensor.matmul(H2, w2tb, h1b, start=False, stop=True)
            t2 = singles.tile([56, MOLS_PER_CORE], f32)
            nc.scalar.activation(out=t2, in_=H2, func=Act.Exp, bias=b2p, scale=1.0)
            h2 = singles.tile([56, MOLS_PER_CORE], f32)
            nc.scalar.activation(out=h2, in_=t2, func=Act.Ln, bias=1.0, scale=1.0)

            H3 = ps_m.tile([1, MOLS_PER_CORE], f32, tag="mf")
            nc.tensor.matmul(H3, w3t_sb, h2, start=True, stop=True)
            outs = singles.tile([1, MOLS_PER_CORE], f32)
            nc.vector.tensor_scalar_add(outs, H3, b3p[0:1, 0:1])
            nc.sync.dma_start(out=out_h[:, :], in_=outs)

    nc.compile()
    return nc


def kernel(feats, segment_ids, W, att_w, W1, b1, W2, b2, W3, b3):
    global LAST_EXEC_NS, LAST_RESULTS

    feats = np.asarray(feats, dtype=np.float32)
    segment_ids = np.asarray(segment_ids, dtype=np.int32)
    W = np.asarray(W, dtype=np.float32)
    att_w = np.asarray(att_w, dtype=np.float32)
    W1 = np.asarray(W1, dtype=np.float32)
    b1 = np.asarray(b1, dtype=np.float32)
    W2 = np.asarray(W2, dtype=np.float32)
    b2 = np.asarray(b2, dtype=np.float32)
    W3 = np.asarray(W3, dtype=np.float32)
    b3 = np.asarray(b3, dtype=np.float32)

    # ---- host-side shard + pad (segment_ids are sorted) ----
    counts = np.bincount(segment_ids, minlength=N_MOL)
    starts = np.zeros(N_MOL + 1, dtype=np.int64)
    np.cumsum(counts, out=starts[1:])

    n_groups_total = N_CORES * GROUPS_PER_CORE
    gcounts = counts.reshape(n_groups_total, GROUP_MOLS).sum(axis=1)
    C = int(-(-int(gcounts.max()) // 128) * 128)
    C = max(C, 128)
    nchunk = C // 128
    BMAX = max(max(_batches(nchunk)), max(_batches(nchunk, first_group=True)))

    # shared (replicated) host-prepped weights
    v = (att_w @ W).astype(np.float32)  # v = W.T @ att_w
    # the attention vector v is folded into the features on the host
    # (F' = feats * v_safe); it is divided back out of the mol_fp weights.
    # |v| is clamped so |W.T/v| <= ~450, always fp16-representable; the
    # clamp is used consistently on both sides so it cancels exactly in
    # mol_fp and only perturbs attention scores by <= |F|*1e-4.
    v = np.where(np.abs(v) < 1e-4, np.where(v < 0, -1e-4, 1e-4), v).astype(np.float32)
    wtp32 = (W.T / v[:, None]).astype(np.float32)
    wtp = np.ascontiguousarray(wtp32.astype(np.float16))
    # W1 folded through the mol_fp weights: H1 = (S'/den) @ u1t
    u1t = np.ascontiguousarray((wtp32 @ W1.T).astype(np.float16))
    w2t = np.ascontiguousarray(W2.T)
    w3t = np.ascontiguousarray(W3.T)
    b1c = np.ascontiguousarray(b1.reshape(170, 1))
    b2p = np.ascontiguousarray((b2 - LOG2 * W2.sum(axis=1)).reshape(56, 1)).astype(np.float32)
    b3p = np.ascontiguousarray((b3 - LOG2 * W3.sum(axis=1)).reshape(1, 1)).astype(np.float32)
    iotaf = np.ascontiguousarray(
        np.broadcast_to(np.arange(128, dtype=np.float16), (128, 128))
    )
    ident = np.eye(128, dtype=np.float32)

    in_maps = []
    for core in range(N_CORES):
        # row layout: [F'(0:256) | 1 | F'(256:512) | 1 | pad pad] so each
        # N=257 half of the segment matmul carries a denominator column
        fp = np.zeros((GROUPS_PER_CORE * C, 516), dtype=np.float16)
        fp[:, 256] = 1.0
        fp[:, 513] = 1.0
        sl = np.full((GROUPS_PER_CORE * C,), -1.0, dtype=np.float32)
        for g in range(GROUPS_PER_CORE):
            mol0 = core * MOLS_PER_CORE + g * GROUP_MOLS
            a0 = starts[mol0]
            a1 = starts[mol0 + GROUP_MOLS]
            cnt = a1 - a0
            fpr = (feats[a0:a1] * v[None, :]).astype(np.float16)
            fp[g * C : g * C + cnt, 0:256] = fpr[:, 0:256]
            fp[g * C : g * C + cnt, 257:513] = fpr[:, 256:512]
            sl[g * C : g * C + cnt] = (segment_ids[a0:a1] - mol0).astype(np.float32)
        fp = np.ascontiguousarray(
            fp.reshape(GROUPS_PER_CORE * C // 128, 128, 516).transpose(1, 0, 2)
        )
        sl = np.ascontiguousarray(
            sl.reshape(GROUPS_PER_CORE * C // 128, 128).transpose(1, 0)
        )
        in_maps.append(
            dict(
                feats=fp,
                segl=sl,
                wtp=wtp,
                u1t=u1t,
                w2t=w2t,
                w3t=w3t,
                b1c=b1c,
                b2p=b2p,
                b3p=b3p,
                iotaf=iotaf,
                ident=ident,
            )
        )

    if nchunk not in _PROGRAM_CACHE:
        _PROGRAM_CACHE[nchunk] = _build_program(nchunk)
    nc = _PROGRAM_CACHE[nchunk]

    from concourse.bass_utils import run_bass_kernel_spmd

    trace = os.environ.get("KERNEL_TRACE", "0") == "1"
    res = run_bass_kernel_spmd(
        nc, in_maps, core_ids=list(range(N_CORES)), trace=trace
    )
    LAST_EXEC_NS = res.exec_time_ns
    LAST_RESULTS = res

    mol_fp = np.concatenate([r["mol_fp"] for r in res.results], axis=0)
    out = np.concatenate([r["out"].reshape(-1) for r in res.results], axis=0)
    return out, mol_fp


# revision 40
# speedup vs baseline: 1.0177x; 1.0027x over previous
"""AttentionPool kernel for Trainium2 (8 NeuronCores, Bass/Tile).

Reference computation:
    wf = feats @ W.T
    scores = leaky_relu((att_w * wf).sum(-1))
    weights = segment_softmax(scores)
    mol_fp = segment_sum(weights[:, None] * wf)
    out = MLP(mol_fp)   (Dense 512->170 ssp, 170->56 ssp, 56->1)

Algebraic restructuring (exact in real arithmetic):
    scores = leaky_relu(feats @ v)        with v = W.T @ att_w
    S      = segment_sum(weights * feats)             [n_mol, feat]
    mol_fp = S @ W.T                                  (linearity of W)
This removes the [131072 x 512 x 512] matmul entirely (32x fewer FLOPs).

Softmax is computed without max-subtraction (scores are O(+-6) so exp is
safe) and without per-atom normalization: the one-hot scatter matrix
carries exp(score); S rows are divided by the denominator after segment
summation.

Per-core segment sums use the PE one-hot trick: for a 128-atom chunk,
lhsT[a, m] = exp(score_a) * (seg_a == m), rhs = the feats chunk,
accumulated in PSUM. Each chunk row is laid out on the host as
[F'(0:256) | 1 | F'(256:512) | 1 | pad] (516 wide) and the segment
matmul runs as two N=257 halves, so column 256 of each PSUM half
accumulates the softmax denominator for free.

The attention vector v is folded into the features on the host
(F' = feats * v, fp16), which turns the per-atom score into a plain row
sum (split between DVE tensor_reduce and ScalarE activation-accumulate);
v is divided back out of the mol_fp weights (W.T / v, with |v| clamped
at 1e-4 so the fp16 weight magnitudes stay bounded; the clamp cancels
exactly in mol_fp and perturbs scores by <= ~1e-4).

Precision: feats / one-hot / segment-matmul / mol_fp-matmul run in fp16
(single-pass PE matmuls -- fp32 is double-pumped LOW_HIGH = 2x cost;
fp16 keeps 11 mantissa bits: measured end-to-end absmax/scale ~4e-4).
All accumulation (PSUM, score reduction) is fp32, and the MLP runs fp32
(the final output is a near-cancellation of O(1) hidden values, so fp16
noise there would dominate).

Sharding: 512 consecutive molecules per core (segment_ids are sorted),
4 groups of 128 molecules per core. The host pads each group's atom
list to a common capacity C (multiple of 128) with zero feats and
seg = -1 (one-hot never matches => zero contribution).

The shifted-softplus "- log(2)" terms are folded into the downstream
biases on the host: b2' = b2 - log2 * W2.sum(1), b3' = b3 - log2 * W3.sum(1).
Device softplus(x) is computed as ln(exp(x) + 1) (the ACT table set
covering exp has no softplus entry; exp/ln/lrelu/copy share one set).
"""

import os

import numpy as np

N_ATOMS = 131072
N_MOL = 4096
FEAT = 512
N_CORES = 8
MOLS_PER_CORE = N_MOL // N_CORES  # 512
GROUPS_PER_CORE = 4
GROUP_MOLS = 128  # one-hot width / PSUM partition dim
LOG2 = float(np.log(2.0))

# Set by kernel() when tracing is enabled via KERNEL_TRACE=1
LAST_EXEC_NS = None
LAST_RESULTS = None

_PROGRAM_CACHE = {}


def _batches(nchunk, first_group=False):
    """Split nchunk 128-atom chunks into sub-batches of <=9 chunks.
    Each sub-batch is one DMA + one reduce. The first group ramps up with
    small batches so the pipeline fills quickly after launch."""
    if first_group:
        sizes = []
        n = nchunk
        for s in (1, 2, 2, 4):
            if n <= 0:
                break
            s = min(s, n)
            sizes.append(s)
            n -= s
        while n > 0:
            s = min(8, n)
            # avoid a tiny trailing batch
            if 0 < n - s < 3 and s == 8:
                s = n - 2
            sizes.append(s)
            n -= s
        return sizes
    nb = -(-nchunk // 9)
    base = nchunk // nb
    rem = nchunk - base * nb
    return [base + 1] * rem + [base] * (nb - rem)


def _build_program(nchunk):
    import concourse.bacc as bacc
    import concourse.mybir as mybir
    import concourse.tile as tile

    f32 = mybir.dt.float32
    f16 = mybir.dt.float16
    Alu = mybir.AluOpType
    Act = mybir.ActivationFunctionType
    Ax = mybir.AxisListType

    C = nchunk * 128  # atoms per group (padded)
    ncol = GROUPS_PER_CORE * nchunk  # total 128-atom chunks per core
    bsizes = _batches(nchunk)
    bsizes0 = _batches(nchunk, first_group=True)
    BMAX = max(max(bsizes), max(bsizes0))

    nc = bacc.Bacc("TRN2")

    feats_h = nc.dram_tensor("feats", [128, GROUPS_PER_CORE * nchunk, 516], f16, kind="ExternalInput")
    segl_h = nc.dram_tensor("segl", [128, GROUPS_PER_CORE * nchunk], f32, kind="ExternalInput")
    wtp_h = nc.dram_tensor("wtp", [FEAT, FEAT], f16, kind="ExternalInput")
    u1t_h = nc.dram_tensor("u1t", [FEAT, 170], f16, kind="ExternalInput")
    w2t_h = nc.dram_tensor("w2t", [170, 56], f32, kind="ExternalInput")
    w3t_h = nc.dram_tensor("w3t", [56, 1], f32, kind="ExternalInput")
    b1_h = nc.dram_tensor("b1c", [170, 1], f32, kind="ExternalInput")
    b2p_h = nc.dram_tensor("b2p", [56, 1], f32, kind="ExternalInput")
    b3p_h = nc.dram_tensor("b3p", [1, 1], f32, kind="ExternalInput")
    iota_h = nc.dram_tensor("iotaf", [128, 128], f16, kind="ExternalInput")
    id32_h = nc.dram_tensor("ident", [128, 128], f32, kind="ExternalInput")

    molfp_h = nc.dram_tensor("mol_fp", [MOLS_PER_CORE, FEAT], f32, kind="ExternalOutput")
    out_h = nc.dram_tensor("out", [1, MOLS_PER_CORE], f32, kind="ExternalOutput")

    with tile.TileContext(nc) as tc:
        with (
            tc.tile_pool(name="singles", bufs=1) as singles,
            tc.tile_pool(name="fpool", bufs=9) as fpool,
            tc.tile_pool(name="scratch", bufs=2) as scratch,
            tc.tile_pool(name="ohpool", bufs=8) as ohpool,
            tc.tile_pool(name="misc", bufs=3) as misc,
            tc.tile_pool(name="stpool", bufs=5) as stpool,
            tc.tile_pool(name="ps_sa", bufs=2, space="PSUM") as ps_sa,
            tc.tile_pool(name="ps_sb", bufs=2, space="PSUM") as ps_sb,
            tc.tile_pool(name="ps_t", bufs=1, space="PSUM") as ps_t,
            tc.tile_pool(name="ps_m", bufs=1, space="PSUM") as ps_m,
            tc.tile_pool(name="ps_h", bufs=1, space="PSUM") as ps_h,
        ):
            # ---- constants / weights (loaded once) ----
            # tiles for weights/constants; the heavier weight DMAs are
            # emitted after the main loop so the startup Sync queue serves
            # the first feats batches first (deps still pull them in time)
            wtp_sb = singles.tile([128, 4, FEAT], f16)
            u1t_sb = singles.tile([128, 4, 170], f16)
            w2ta = singles.tile([128, 56], f32)
            w2tb = singles.tile([42, 56], f32)
            w3t_sb = singles.tile([56, 1], f32)
            b1a = singles.tile([128, 1], f32)
            b1b = singles.tile([42, 1], f32)
            b2p = singles.tile([56, 1], f32)
            b3p = singles.tile([1, 1], f32)
            id32_sb = singles.tile([128, 128], f32)
            iota_sb = singles.tile([128, 128], f16)
            nc.sync.dma_start(out=iota_sb, in_=iota_h[:, :])
            seg_all = singles.tile([128, ncol], f32)
            nc.sync.dma_start(out=seg_all, in_=segl_h[:, :])

            def load_weights_late():
                nc.sync.dma_start(out=wtp_sb, in_=wtp_h[:, :].rearrange("(c p) j -> p c j", p=128))
                nc.sync.dma_start(out=u1t_sb, in_=u1t_h[:, :].rearrange("(c p) k -> p c k", p=128))
                nc.sync.dma_start(out=w2ta, in_=w2t_h[0:128, :])
                nc.sync.dma_start(out=w2tb, in_=w2t_h[128:170, :])
                nc.sync.dma_start(out=w3t_sb, in_=w3t_h[:, :])
                nc.sync.dma_start(out=b1a, in_=b1_h[0:128, :])
                nc.sync.dma_start(out=b1b, in_=b1_h[128:170, :])
                nc.sync.dma_start(out=b2p, in_=b2p_h[:, :])
                nc.sync.dma_start(out=b3p, in_=b3p_h[:, :])
                nc.sync.dma_start(out=id32_sb, in_=id32_h[:, :])

            scores_all = singles.tile([128, ncol], f32)
            lr_all = singles.tile([128, ncol], f32)
            ex_all = singles.tile([128, ncol], f32)
            # first-MLP-layer accumulators, filled per group directly from
            # the transposed-S tiles via U1 = (W.T/v) @ W1.T folded on host
            H1a = ps_h.tile([128, MOLS_PER_CORE], f32, tag="h1a")
            H1b = ps_h.tile([42, MOLS_PER_CORE], f32, tag="h1b")

            F_slices = {}  # col -> (tile, idx within batch)
            gbatch = 0
            weights_loaded = [False]
            for g in range(GROUPS_PER_CORE):
                # ---- phase 1: stream feats sub-batches; scores = rowsum(F')
                # (features are pre-scaled by v on the host). Reduce work is
                # split between DVE (tensor_reduce, batched) and the Scalar
                # engine (activation Copy with accumulate, per chunk).
                b0 = 0
                for bsz in (bsizes0 if g == 0 else bsizes):
                    col0 = g * nchunk + b0
                    # 516-wide chunk rows: [F(0:256) | 1 | F(256:512) | 1 | pad]
                    # so each half of the segment matmul (N=257) carries the
                    # softmax denominator in its ones column. DRAM layout is
                    # partition-major: one contiguous run per partition per DMA.
                    F = fpool.tile([128, bsz, 516], f16, tag="F")
                    nc.sync.dma_start(out=F, in_=feats_h[:, col0 : col0 + bsz, :])
                    fdat = F[:, :, 0:514].rearrange("p c (h e) -> p c h e", h=2)[
                        :, :, :, 0:256
                    ]
                    # split each batch's reduction DVE/ACT so both engines
                    # work concurrently -- keeps the serial latency before
                    # this batch's exp (and the PE's one-hot supply) short
                    h = max(1, (bsz * 5 + 4) // 9) if bsz > 1 else 1
                    part = scratch.tile([128, BMAX, 2], f32, tag="part")
                    nc.vector.tensor_reduce(
                        out=part[:, :h, :],
                        in_=fdat[:, :h, :, :],
                        axis=Ax.X,
                        op=Alu.add,
                    )
                    nc.vector.tensor_tensor(
                        out=scores_all[:, col0 : col0 + h],
                        in0=part[:, :h, 0],
                        in1=part[:, :h, 1],
                        op=Alu.add,
                    )
                    # lrelu+exp for the DVE half immediately, so its one-hots
                    # release while the Scalar engine still reduces its half
                    bh = slice(col0, col0 + h)
                    nc.vector.scalar_tensor_tensor(
                        out=lr_all[:, bh], in0=scores_all[:, bh], scalar=0.01,
                        in1=scores_all[:, bh], op0=Alu.mult, op1=Alu.max,
                    )
                    nc.scalar.activation(out=ex_all[:, bh], in_=lr_all[:, bh], func=Act.Exp)
                    for j in range(h, bsz):
                        scr = scratch.tile([128, FEAT], f16, tag="scr")
                        nc.scalar.activation(
                            out=scr,
                            in_=fdat[:, j, :, :],
                            func=Act.Copy,
                            bias=0.0,
                            scale=1.0,
                            accum_out=scores_all[:, col0 + j : col0 + j + 1],
                        )
                    # leaky_relu (DVE, avoids an ACT table swap) + exp per
                    # batch so phase 2 can start before the group completes
                    if h < bsz:
                        bb = slice(col0 + h, col0 + bsz)
                        nc.vector.scalar_tensor_tensor(
                            out=lr_all[:, bb],
                            in0=scores_all[:, bb],
                            scalar=0.01,
                            in1=scores_all[:, bb],
                            op0=Alu.mult,
                            op1=Alu.max,
                        )
                        nc.scalar.activation(out=ex_all[:, bb], in_=lr_all[:, bb], func=Act.Exp)
                    for j in range(bsz):
                        F_slices[col0 + j] = (F, j)
                    b0 += bsz
                    gbatch += 1
                    if not weights_loaded[0]:
                        weights_loaded[0] = True
                        load_weights_late()

                # ---- phase 2: one-hot segment-sum matmuls (two N=257
                # halves; col 256 of each accumulates the denominator) ----
                S_a = ps_sa.tile([128, 257], f32, tag="sa")
                S_b = ps_sb.tile([128, 257], f32, tag="sb")
                for c in range(nchunk):
                    col = g * nchunk + c
                    oh = ohpool.tile([128, 128], f16, tag="oh")
                    # oh[a, m] = (iota[a, m] == seg[a]) * ex[a]
                    nc.vector.tensor_scalar(
                        out=oh,
                        in0=iota_sb,
                        scalar1=seg_all[:, col : col + 1],
                        scalar2=ex_all[:, col : col + 1],
                        op0=Alu.is_equal,
                        op1=Alu.mult,
                    )
                    Ft, j = F_slices.pop(col)
                    nc.tensor.matmul(
                        S_a, oh, Ft[:, j, 0:257], start=(c == 0), stop=(c == nchunk - 1)
                    )
                    nc.tensor.matmul(
                        S_b, oh, Ft[:, j, 257:514], start=(c == 0), stop=(c == nchunk - 1)
                    )

                # ---- phase 3: normalize, mol_fp = S @ W.T ----
                denc = misc.tile([128, 1], f32, tag="denc")
                nc.vector.tensor_scalar_max(denc, S_a[:, 256:257], 1e-30)
                recip = misc.tile([128, 1], f32, tag="recip")
                nc.vector.reciprocal(recip, denc)
                S_sb = misc.tile([128, FEAT], f32, tag="ssb")
                nc.vector.tensor_scalar_mul(S_sb[:, 0:256], S_a[:, 0:256], recip)
                nc.vector.tensor_scalar_mul(S_sb[:, 256:512], S_b[:, 0:256], recip)

                MF = ps_m.tile([128, FEAT], f32, tag="mf")
                STs = []
                for ic in range(4):
                    STp = ps_t.tile([128, 128], f32, tag="tr")
                    nc.tensor.transpose(STp, S_sb[:, ic * 128 : (ic + 1) * 128], id32_sb)
                    ST = stpool.tile([128, 128], f16, tag="st")
                    nc.scalar.copy(out=ST, in_=STp)
                    STs.append(ST)
                    nc.tensor.matmul(
                        MF, ST, wtp_sb[:, ic, :], start=(ic == 0), stop=(ic == 3)
                    )
                mf_sb = misc.tile([128, FEAT], f32, tag="mfsb")
                nc.vector.tensor_copy(mf_sb, MF)
                nc.sync.dma_start(out=molfp_h[g * 128 : (g + 1) * 128, :], in_=mf_sb)

                # first MLP layer accumulates per group from the ST tiles:
                # H1[:, g-slice] = sum_ic u1t[ic].T @ ST[ic]
                ms = slice(g * 128, (g + 1) * 128)
                for ic in range(4):
                    nc.tensor.matmul(
                        H1a[:, ms], u1t_sb[:, ic, 0:128], STs[ic],
                        start=(ic == 0), stop=(ic == 3),
                    )
                for ic in range(4):
                    nc.tensor.matmul(
                        H1b[:, ms], u1t_sb[:, ic, 128:170], STs[ic],
                        start=(ic == 0), stop=(ic == 3),
                    )

            # ---- MLP over all 512 molecules of this core ----
            # softplus(x + b) computed as ln(exp(x + b) + 1); exp's grouped
            # before ln's to minimize ACT function-table reloads
            t1a = singles.tile([128, MOLS_PER_CORE], f32)
            nc.scalar.activation(out=t1a, in_=H1a, func=Act.Exp, bias=b1a, scale=1.0)
            t1b = singles.tile([42, MOLS_PER_CORE], f32)
            nc.scalar.activation(out=t1b, in_=H1b, func=Act.Exp, bias=b1b, scale=1.0)
            h1a = singles.tile([128, MOLS_PER_CORE], f32)
            nc.scalar.activation(out=h1a, in_=t1a, func=Act.Ln, bias=1.0, scale=1.0)
            h1b = singles.tile([42, MOLS_PER_CORE], f32)
            nc.scalar.activation(out=h1b, in_=t1b, func=Act.Ln, bias=1.0, scale=1.0)

            H2 = ps_m.tile([56, MOLS_PER_CORE], f32, tag="mf")
            nc.tensor.matmul(H2, w2ta, h1a, start=True, stop=False)
            nc.tensor.matmul(H2, w2tb, h1b, start=False, stop=True)
            t2 = singles.tile([56, MOLS_PER_CORE], f32)
            nc.scalar.activation(out=t2, in_=H2, func=Act.Exp, bias=b2p, scale=1.0)
            h2 = singles.tile([56, MOLS_PER_CORE], f32)
            nc.scalar.activation(out=h2, in_=t2, func=Act.Ln, bias=1.0, scale=1.0)

            H3 = ps_m.tile([1, MOLS_PER_CORE], f32, tag="mf")
            nc.tensor.matmul(H3, w3t_sb, h2, start=True, stop=True)
            outs = singles.tile([1, MOLS_PER_CORE], f32)
            nc.vector.tensor_scalar_add(outs, H3, b3p[0:1, 0:1])
            nc.sync.dma_start(out=out_h[:, :], in_=outs)

    nc.compile()
    return nc


def kernel(feats, segment_ids, W, att_w, W1, b1, W2, b2, W3, b3):
    global LAST_EXEC_NS, LAST_RESULTS

    feats = np.asarray(feats, dtype=np.float32)
    segment_ids = np.asarray(segment_ids, dtype=np.int32)
    W = np.asarray(W, dtype=np.float32)
    att_w = np.asarray(att_w, dtype=np.float32)
    W1 = np.asarray(W1, dtype=np.float32)
    b1 = np.asarray(b1, dtype=np.float32)
    W2 = np.asarray(W2, dtype=np.float32)
    b2 = np.asarray(b2, dtype=np.float32)
    W3 = np.asarray(W3, dtype=np.float32)
    b3 = np.asarray(b3, dtype=np.float32)

    # ---- host-side shard + pad (segment_ids are sorted) ----
    counts = np.bincount(segment_ids, minlength=N_MOL)
    starts = np.zeros(N_MOL + 1, dtype=np.int64)
    np.cumsum(counts, out=starts[1:])

    n_groups_total = N_CORES * GROUPS_PER_CORE
    gcounts = counts.reshape(n_groups_total, GROUP_MOLS).sum(axis=1)
    C = int(-(-int(gcounts.max()) // 128) * 128)
    C = max(C, 128)
    nchunk = C // 128
    BMAX = max(max(_batches(nchunk)), max(_batches(nchunk, first_group=True)))

    # shared (replicated) host-prepped weights
    v = (att_w @ W).astype(np.float32)  # v = W.T @ att_w
    # the attention vector v is folded into the features on the host
    # (F' = feats * v_safe); it is divided back out of the mol_fp weights.
    # |v| is clamped so |W.T/v| <= ~450, always fp16-representable; the
    # clamp is used consistently on both sides so it cancels exactly in
    # mol_fp and only perturbs attention scores by <= |F|*1e-4.
    v = np.where(np.abs(v) < 1e-4, np.where(v < 0, -1e-4, 1e-4), v).astype(np.float32)
    wtp32 = (W.T / v[:, None]).astype(np.float32)
    wtp = np.ascontiguousarray(wtp32.astype(np.float16))
    # W1 folded through the mol_fp weights: H1 = (S'/den) @ u1t
    u1t = np.ascontiguousarray((wtp32 @ W1.T).astype(np.float16))
    w2t = np.ascontiguousarray(W2.T)
    w3t = np.ascontiguousarray(W3.T)
    b1c = np.ascontiguousarray(b1.reshape(170, 1))
    b2p = np.ascontiguousarray((b2 - LOG2 * W2.sum(axis=1)).reshape(56, 1)).astype(np.float32)
    b3p = np.ascontiguousarray((b3 - LOG2 * W3.sum(axis=1)).reshape(1, 1)).astype(np.float32)
    iotaf = np.ascontiguousarray(
        np.broadcast_to(np.arange(128, dtype=np.float16), (128, 128))
    )
    ident = np.eye(128, dtype=np.float32)

    in_maps = []
    for core in range(N_CORES):
        # row layout: [F'(0:256) | 1 | F'(256:512) | 1 | pad pad] so each
        # N=257 half of the segment matmul carries a denominator column
        fp = np.zeros((GROUPS_PER_CORE * C, 516), dtype=np.float16)
        fp[:, 256] = 1.0
        fp[:, 513] = 1.0
        sl = np.full((GROUPS_PER_CORE * C,), -1.0, dtype=np.float32)
        for g in range(GROUPS_PER_CORE):
            mol0 = core * MOLS_PER_CORE + g * GROUP_MOLS
            a0 = starts[mol0]
            a1 = starts[mol0 + GROUP_MOLS]
            cnt = a1 - a0
            fpr = (feats[a0:a1] * v[None, :]).astype(np.float16)
            fp[g * C : g * C + cnt, 0:256] = fpr[:, 0:256]
            fp[g * C : g * C + cnt, 257:513] = fpr[:, 256:512]
            sl[g * C : g * C + cnt] = (segment_ids[a0:a1] - mol0).astype(np.float32)
        fp = np.ascontiguousarray(
            fp.reshape(GROUPS_PER_CORE * C // 128, 128, 516).transpose(1, 0, 2)
        )
        sl = np.ascontiguousarray(
            sl.reshape(GROUPS_PER_CORE * C // 128, 128).transpose(1, 0)
        )
        in_maps.append(
            dict(
                feats=fp,
                segl=sl,
                wtp=wtp,
                u1t=u1t,
                w2t=w2t,
                w3t=w3t,
                b1c=b1c,
                b2p=b2p,
                b3p=b3p,
                iotaf=iotaf,
                ident=ident,
            )
        )

    if nchunk not in _PROGRAM_CACHE:
        _PROGRAM_CACHE[nchunk] = _build_program(nchunk)
    nc = _PROGRAM_CACHE[nchunk]

    from concourse.bass_utils import run_bass_kernel_spmd

    trace = os.environ.get("KERNEL_TRACE", "0") == "1"
    res = run_bass_kernel_spmd(
        nc, in_maps, core_ids=list(range(N_CORES)), trace=trace
    )
    LAST_EXEC_NS = res.exec_time_ns
    LAST_RESULTS = res

    mol_fp = np.concatenate([r["mol_fp"] for r in res.results], axis=0)
    out = np.concatenate([r["out"].reshape(-1) for r in res.results], axis=0)
    return out, mol_fp
